# revision 22
# baseline (speedup 1.0000x reference)
"""Bass/Trainium2 kernel for nn_DecodeWrapperEager (sparse paged-attention decode).

Problem (hardcoded): B=8, Hq=32, Hk=8, D=64, S=32, NB=128, T=4096, W=1024.
One decode step of GQA attention with a paged KV cache, sliding window W and
a sink logit.  Output: [B, 1, Hq, D] float32.

Sharding: KV heads across the 8 cores (tensor-parallel).  Each core receives
its head's cache in two layouts staged on host (fp16: same PE speed and DMA
bytes as bf16, 8x the mantissa):
  - ktc [64, PL+pad]   : K^T, column = page*32 + slot (page-linear), k_last
                         scattered at the decode position.
  - vc4 [PL+pad, 260]  : row t = [Vaug_t | Vaug_{t+128} | Vaug_{t+256} |
                         Vaug_{t+384}] where Vaug_t = [V_t | 1] (the fused
                         ones-column makes the softmax denominator fall out
                         of the P @ [V|1] matmul).  Position-quadded so one
                         DMA descriptor covers four 128-chunks (520B runs).
Per-batch sliding-window offsets are identical on every core, so they are
baked into the single SPMD program; the device gathers only the window pages.
Batches are packed in pairs onto the 128-partition contraction dim (head dim
is 64) so the PE array and all 16 DMA ports are fully used.  Windows are
loaded to ceil-of-128 chunks (overreads land in padded/neighbor pages); the
out-of-window tail rows are excluded by partition-sliced tail matmuls.  The
two P@[V|1] matmuls per chunk use 4-wide stationaries so both batches'
accumulators live on partitions 0-3 (compute engines need 32-aligned
partition starts, so an [8,130] result would need a DMA round trip).
Engine assignment is by pipeline stage, and all loads are emitted before all
compute so every queue is monotonic in dependency time: sync=K,
gpsimd=V(A half)+outputs, scalar=V(B half)+exp, vector=epilogue, PE=matmuls.
"""

import os
import numpy as np

import concourse.bacc as bacc
import concourse.tile as tile
import concourse.mybir as mybir
from concourse.bass_utils import run_bass_kernel_spmd

F32 = mybir.dt.float32
F16 = mybir.dt.float16
F16_NP = np.float16

B, Hq, Hk, D = 8, 32, 8, 64
S, NB = 32, 128
T = NB * S          # 4096 positions per sequence
P = B * NB          # 1024 cache pages
W = 1024            # sliding window
G = Hq // Hk        # 4 query heads per kv head
NCORES = 8
PL = P * S          # 32768 page-linear positions
KPAD = 128          # ceil-chunk overread slack past the last page
VPAD = 1024         # slack for the position-shifted right slots + overread
DV = D + 1          # V row with fused ones-column

LAST_EXEC_NS = None


def _window_runs(sbi_b, pos):
    """Contiguous runs (col, len) in page-linear space covering [lo, pos]."""
    lo = max(0, pos - W + 1)
    nv = pos + 1 - lo
    runs = []
    t = lo
    while t <= pos:
        blk = t // S
        s0 = t % S
        s1 = min(S - 1, pos - blk * S)
        col = int(sbi_b[blk]) * S + s0
        ln = s1 - s0 + 1
        if runs and runs[-1][0] + runs[-1][1] == col:
            runs[-1][1] += ln
        else:
            runs.append([col, ln])
        t = blk * S + s1 + 1
    return lo, nv, [(c, l) for c, l in runs]


def _pairs_of(win):
    # larger window first in each pair: tail-chunk matmuls rely on
    # nch[bA] >= nch[bB]
    order = sorted(range(B), key=lambda b: -win[b][1])
    return [(order[i], order[i + 1]) for i in range(0, B, 2)]


def _emit_v_load(nc, vt, vc4, hbase, b, win, nch, veng):
    """DMA batch b's V window (ceil-chunks) into vt cols [hbase, ...)."""
    lo, nv, runs = win[b]
    nload = nch[b] * 128
    if len(runs) == 1:
        # quad/pair fast path: one descriptor covers 4 (or 2) chunks
        col = runs[0][0]
        nquad, rest = divmod(nch[b], 4)
        if nquad:
            dst = vt[:, hbase:hbase + nquad * 4 * DV].rearrange(
                "p (cq f) -> p cq f", f=4 * DV)
            src = vc4[col:col + nquad * 512, :].rearrange(
                "(cq q) f -> q cq f", q=512)[0:128]
            veng.dma_start(dst, src)
        c0 = nquad * 4
        if rest >= 2:
            dst = vt[:, hbase + c0 * DV:hbase + (c0 + 2) * DV]
            src = vc4[col + c0 * 128:col + c0 * 128 + 128, 0:2 * DV]
            veng.dma_start(dst, src)
            c0 += 2
            rest -= 2
        if rest:
            veng.dma_start(
                vt[:, hbase + c0 * DV:hbase + (c0 + 1) * DV],
                vc4[col + c0 * 128:col + c0 * 128 + 128, 0:DV],
            )
    else:
        # general multi-run path (non-arange page tables)
        o = 0
        for ri, (col, ln) in enumerate(runs):
            if ri == len(runs) - 1:
                ln += nload - nv
            while ln > 0:
                c, p0 = divmod(o, 128)
                cbase = hbase + c * DV
                if p0 == 0 and ln >= 128:
                    nf = ln // 128
                    if nf > 1:
                        dst = vt[:, hbase:hbase + nch[b] * DV].rearrange(
                            "p (c f) -> p c f", f=DV)[:, c:c + nf, :]
                        src = vc4[col:col + nf * 128, 0:DV].rearrange(
                            "(c p) f -> p c f", p=128)
                    else:
                        dst = vt[:, cbase:cbase + DV]
                        src = vc4[col:col + 128, 0:DV]
                    veng.dma_start(dst, src)
                    adv = nf * 128
                else:
                    seg = min(128 - p0, ln)
                    veng.dma_start(
                        vt[p0:p0 + seg, cbase:cbase + DV],
                        vc4[col:col + seg, 0:DV],
                    )
                    adv = seg
                o += adv
                col += adv
                ln -= adv


def _build_program(win):
    """win: list per batch of (lo, nv, runs). Returns finalized Bacc program.

    The program is shared by all 8 cores (SPMD); only the data differs.
    """
    nc = bacc.Bacc("TRN2", debug=False)

    ktc = nc.dram_tensor("ktc", [D, PL + KPAD], F16, kind="ExternalInput")
    vc4 = nc.dram_tensor("vc4", [PL + VPAD, 4 * DV], F16, kind="ExternalInput")
    qtz = nc.dram_tensor("qtz", [128, B * G], F16, kind="ExternalInput")
    sk = nc.dram_tensor("sk", [G, 1], F32, kind="ExternalInput")
    out = nc.dram_tensor("out", [4, G, 2 * D], F32, kind="ExternalOutput")

    pairs = _pairs_of(win)
    nch = {b: (win[b][1] + 127) // 128 for b in range(B)}
    rem = {b: win[b][1] - (nch[b] - 1) * 128 for b in range(B)}

    with tile.TileContext(nc) as tc:
        with (
            tc.tile_pool(name="const", bufs=1) as cpool,
            tc.tile_pool(name="kt", bufs=4) as kpool,
            tc.tile_pool(name="vv", bufs=4) as vpool,
            tc.tile_pool(name="ee", bufs=4) as epool,
            tc.tile_pool(name="small", bufs=8) as spool,
            tc.tile_pool(name="outp", bufs=4) as opool,
            tc.tile_pool(name="qkp", bufs=4, space="PSUM") as qkpool,
            tc.tile_pool(name="pvp", bufs=2, space="PSUM") as pvpool,
        ):
            qtz_sb = cpool.tile([128, B * G], F16)
            nc.scalar.dma_start(qtz_sb[:], qtz[:])
            sk_sb = cpool.tile([G, 1], F32)
            nc.scalar.dma_start(sk_sb[:], sk[:])
            esink = cpool.tile([G, 1], F32)
            nc.scalar.activation(esink[:], sk_sb[:], mybir.ActivationFunctionType.Exp)

            # ---- load phase: all K and V DMAs up front ----
            ktiles, vtiles = [], []
            for pi, (bA, bB) in enumerate(pairs):
                ncp = nch[bA]
                ktile = kpool.tile([128, ncp * 128], F16, tag="ktile")
                # half-grouped V: A slots [0, ncp*DV), B slots [ncp*DV, ...)
                vt = vpool.tile([128, 2 * ncp * DV], F16, tag="vtile")
                ktiles.append(ktile)
                vtiles.append(vt)
                for half, b in ((0, bA), (1, bB)):
                    lo, nv, runs = win[b]
                    nload = nch[b] * 128
                    o = 0
                    for ri, (col, ln) in enumerate(runs):
                        if ri == len(runs) - 1:
                            ln += nload - nv
                        if pi == 0 and len(runs) == 1 and ln > 512:
                            # split so the first chunks' receipt lands sooner
                            nc.sync.dma_start(
                                ktile[half * 64:half * 64 + 64, o:o + 512],
                                ktc[:, col:col + 512],
                            )
                            nc.sync.dma_start(
                                ktile[half * 64:half * 64 + 64, o + 512:o + ln],
                                ktc[:, col + 512:col + ln],
                            )
                        else:
                            nc.sync.dma_start(
                                ktile[half * 64:half * 64 + 64, o:o + ln],
                                ktc[:, col:col + ln],
                            )
                        o += ln
                    _emit_v_load(nc, vt, vc4, half * ncp * DV, b, win, nch,
                                 nc.gpsimd)

            # ---- compute phase ----
            for pi, (bA, bB) in enumerate(pairs):
                ncp = nch[bA]
                ktile, vt = ktiles[pi], vtiles[pi]
                qk = qkpool.tile([128, ncp * 2 * G], F32)
                for c in range(ncp):
                    nc.tensor.matmul(
                        qk[:, c * 2 * G:(c + 1) * 2 * G],
                        ktile[:, c * 128:(c + 1) * 128],
                        qtz_sb[:, pi * 2 * G:(pi + 1) * 2 * G],
                        start=True, stop=True, skip_group_check=True,
                    )
                e = epool.tile([128, ncp * 2 * G], F16)
                nc.scalar.activation(
                    e[:], qk[:], mybir.ActivationFunctionType.Exp,
                    scale=1.0 / 8.0,
                )
                # P @ [V|1] with 4-wide stationaries: batch A accumulates in
                # pv[0:4, 0:65], batch B in pv[0:4, 65:130]; tail chunks
                # slice the row count to the window
                nA, nB = nch[bA], nch[bB]
                last_pair = pi == len(pairs) - 1
                vvi = vt.rearrange("p (h c f) -> p c h f", h=2, f=DV)
                if not last_pair:
                    # fused [8,130] accumulator: half the PV matmuls; the
                    # partition-4 quadrant is shifted to partition 0 by a
                    # small DMA hop that overlaps later pairs' compute
                    pvF = pvpool.tile([2 * G, 2 * DV], F32, tag="pvF", bufs=2)
                    for c in range(ncp):
                        ec8 = e[:, c * 2 * G:(c + 1) * 2 * G]
                        fullA = c < nA - 1 or rem[bA] == 128
                        fullB = c < nB - 1 or (c == nB - 1 and rem[bB] == 128)
                        if fullA and fullB and c <= nB - 1:
                            nc.tensor.matmul(
                                pvF[:], ec8, vvi[:, c],
                                start=(c == 0), stop=(c == ncp - 1),
                                skip_group_check=True,
                            )
                        else:
                            ra = 128 if fullA else rem[bA]
                            nc.tensor.matmul(
                                pvF[:, 0:DV], ec8[0:ra, :],
                                vt[0:ra, c * DV:(c + 1) * DV],
                                start=(c == 0), stop=(c == ncp - 1),
                                skip_group_check=True,
                            )
                            if c <= nB - 1:
                                rb = 128 if fullB else rem[bB]
                                nc.tensor.matmul(
                                    pvF[:, DV:2 * DV], ec8[0:rb, :],
                                    vt[0:rb, (ncp + c) * DV:(ncp + c + 1) * DV],
                                    start=(c == 0), stop=(c == nB - 1),
                                    skip_group_check=True,
                                )
                    pvsb = spool.tile([2 * G, 2 * DV], F32, tag="pvsb")
                    nc.vector.tensor_copy(pvsb[:], pvF[:])
                    pvb = spool.tile([G, DV], F32, tag="pvb")
                    nc.gpsimd.dma_start(pvb[:], pvsb[G:2 * G, DV:2 * DV])
                    pvhalves = (pvsb[0:G, 0:DV], pvb[:])
                else:
                    # last pair: two 4-wide accumulators keep both batches on
                    # partitions 0-3 - no hop on the critical tail
                    pvA = pvpool.tile([G, DV], F32, tag="pvA", bufs=1)
                    pvB = pvpool.tile([G, DV], F32, tag="pvB", bufs=1)
                    for c in range(ncp):
                        ra = 128 if (c < nA - 1 or rem[bA] == 128) else rem[bA]
                        nc.tensor.matmul(
                            pvA[:],
                            e[0:ra, c * 2 * G:c * 2 * G + G],
                            vt[0:ra, c * DV:(c + 1) * DV],
                            start=(c == 0), stop=(c == ncp - 1),
                            skip_group_check=True,
                        )
                        if c <= nB - 1:
                            rb = 128 if (c < nB - 1 or rem[bB] == 128) else rem[bB]
                            nc.tensor.matmul(
                                pvB[:],
                                e[0:rb, c * 2 * G + G:(c + 1) * 2 * G],
                                vt[0:rb, (ncp + c) * DV:(ncp + c + 1) * DV],
                                start=(c == 0), stop=(c == nB - 1),
                                skip_group_check=True,
                            )
                    pvhalves = (pvA[:], pvB[:])

                osb = opool.tile([G, 2 * D], F32)
                for half in (0, 1):
                    pvh = pvhalves[half]
                    den = spool.tile([G, 1], F32, tag="den")
                    nc.vector.tensor_add(den[:], pvh[:, D:DV], esink[:])
                    inv = spool.tile([G, 1], F32, tag="inv")
                    nc.vector.reciprocal(inv[:], den[:])
                    nc.vector.tensor_scalar_mul(
                        osb[:, half * D:(half + 1) * D], pvh[:, 0:D], inv[:])
                nc.gpsimd.dma_start(out[pi], osb[:])

    nc.finalize()
    return nc


_PROGRAM_CACHE = {}


def kernel(q_last, k_last, v_last, cache_state, seq_block_ids, start_positions,
           sink):
    global LAST_EXEC_NS

    q_last = np.asarray(q_last, dtype=np.float32)
    k_last = np.asarray(k_last, dtype=np.float32)
    v_last = np.asarray(v_last, dtype=np.float32)
    cache_state = np.asarray(cache_state, dtype=np.float32)
    sbi = np.asarray(seq_block_ids)
    pos = np.asarray(start_positions)
    sink = np.asarray(sink, dtype=np.float32)

    win = [_window_runs(sbi[b], int(pos[b])) for b in range(B)]

    key = (tuple(int(p) for p in pos), sbi.tobytes())
    if key not in _PROGRAM_CACHE:
        _PROGRAM_CACHE.clear()
        _PROGRAM_CACHE[key] = _build_program(win)
    nc = _PROGRAM_CACHE[key]

    pairs = _pairs_of(win)

    # decode-token scatter targets (page-linear)
    scat = [int(sbi[b, int(pos[b]) // S]) * S + int(pos[b]) % S for b in range(B)]

    in_maps = []
    for h in range(Hk):
        kt = np.zeros((D, PL + KPAD), dtype=F16_NP)
        kt[:, :PL] = cache_state[:, 0, h].reshape(PL, D).T.astype(F16_NP)
        vcv = np.zeros((PL + VPAD, DV), dtype=F16_NP)
        vcv[:PL, :D] = cache_state[:, 1, h].reshape(PL, D).astype(F16_NP)
        vcv[:, D] = 1.0
        for b in range(B):
            kt[:, scat[b]] = k_last[b, 0, h].astype(F16_NP)
            vcv[scat[b], :D] = v_last[b, 0, h].astype(F16_NP)
        vc4 = np.zeros((PL + VPAD, 4 * DV), dtype=F16_NP)
        vc4[:, 0:DV] = vcv
        vc4[:-128, DV:2 * DV] = vcv[128:]
        vc4[:-256, 2 * DV:3 * DV] = vcv[256:]
        vc4[:-384, 3 * DV:4 * DV] = vcv[384:]
        qz = np.zeros((128, B * G), dtype=F16_NP)
        for pi, (bA, bB) in enumerate(pairs):
            qz[0:D, pi * 2 * G:pi * 2 * G + G] = \
                q_last[bA, 0, h * G:(h + 1) * G].T.astype(F16_NP)
            qz[D:128, pi * 2 * G + G:(pi + 1) * 2 * G] = \
                q_last[bB, 0, h * G:(h + 1) * G].T.astype(F16_NP)
        skv = np.ascontiguousarray(sink[h * G:(h + 1) * G].reshape(G, 1))
        in_maps.append({"ktc": kt, "vc4": vc4, "qtz": qz, "sk": skv})

    prof = os.environ.get("BASS_KERNEL_PROF", "") == "1"
    kwargs = {}
    if prof:
        kwargs = dict(trace=True, trace_cores=list(range(NCORES)))
        tdir = os.environ.get("BASS_KERNEL_PROF_DIR")
        if tdir:
            kwargs["tmpdir"] = tdir
    res = run_bass_kernel_spmd(nc, in_maps, list(range(NCORES)), **kwargs)
    if prof:
        LAST_EXEC_NS = res.exec_time_ns

    full = np.empty((B, 1, Hq, D), dtype=np.float32)
    for h in range(Hk):
        o = res.results[h]["out"]          # [4, G, 2*D]
        for pi, (bA, bB) in enumerate(pairs):
            full[bA, 0, h * G:(h + 1) * G, :] = o[pi, :, 0:D]
            full[bB, 0, h * G:(h + 1) * G, :] = o[pi, :, D:2 * D]
    return full


# revision 23
# speedup vs baseline: 1.0578x; 1.0578x over previous
"""Bass/Trainium2 kernel for nn_DecodeWrapperEager (sparse paged-attention decode).

Problem (hardcoded): B=8, Hq=32, Hk=8, D=64, S=32, NB=128, T=4096, W=1024.
One decode step of GQA attention with a paged KV cache, sliding window W and
a sink logit.  Output: [B, 1, Hq, D] float32.

Sharding: KV heads across the 8 cores (tensor-parallel).  Each core receives
its head's cache in two layouts staged on host (fp16: same PE speed and DMA
bytes as bf16, 8x the mantissa):
  - ktc [64, PL+pad]   : K^T, column = page*32 + slot (page-linear), k_last
                         scattered at the decode position.
  - vc4 [PL+pad, 260]  : row t = [Vaug_t | Vaug_{t+128} | Vaug_{t+256} |
                         Vaug_{t+384}] where Vaug_t = [V_t | 1] (the fused
                         ones-column makes the softmax denominator fall out
                         of the P @ [V|1] matmul).  Position-quadded so one
                         DMA descriptor covers four 128-chunks (520B runs).
Per-batch sliding-window offsets are identical on every core, so they are
baked into the single SPMD program; the device gathers only the window pages.
Batches are packed in pairs onto the 128-partition contraction dim (head dim
is 64) so the PE array and all 16 DMA ports are fully used.  Windows are
loaded to ceil-of-128 chunks (overreads land in padded/neighbor pages); the
out-of-window tail rows are excluded by partition-sliced tail matmuls.  The
two P@[V|1] matmuls per chunk use 4-wide stationaries so both batches'
accumulators live on partitions 0-3 (compute engines need 32-aligned
partition starts, so an [8,130] result would need a DMA round trip).
Engine assignment is by pipeline stage, and all loads are emitted before all
compute so every queue is monotonic in dependency time: sync=K,
gpsimd=V(A half)+outputs, scalar=V(B half)+exp, vector=epilogue, PE=matmuls.
"""

import os
import numpy as np

import concourse.bacc as bacc
import concourse.tile as tile
import concourse.mybir as mybir
from concourse.bass_utils import run_bass_kernel_spmd

F32 = mybir.dt.float32
F16 = mybir.dt.float16
F16_NP = np.float16

B, Hq, Hk, D = 8, 32, 8, 64
S, NB = 32, 128
T = NB * S          # 4096 positions per sequence
P = B * NB          # 1024 cache pages
W = 1024            # sliding window
G = Hq // Hk        # 4 query heads per kv head
NCORES = 8
PL = P * S          # 32768 page-linear positions
KPAD = 128          # ceil-chunk overread slack past the last page
VPAD = 1024         # slack for the position-shifted right slots + overread
DV = D + 1          # V row with fused ones-column

LAST_EXEC_NS = None


def _window_runs(sbi_b, pos):
    """Contiguous runs (col, len) in page-linear space covering [lo, pos]."""
    lo = max(0, pos - W + 1)
    nv = pos + 1 - lo
    runs = []
    t = lo
    while t <= pos:
        blk = t // S
        s0 = t % S
        s1 = min(S - 1, pos - blk * S)
        col = int(sbi_b[blk]) * S + s0
        ln = s1 - s0 + 1
        if runs and runs[-1][0] + runs[-1][1] == col:
            runs[-1][1] += ln
        else:
            runs.append([col, ln])
        t = blk * S + s1 + 1
    return lo, nv, [(c, l) for c, l in runs]


def _pairs_of(win):
    # larger window first in each pair: tail-chunk matmuls rely on
    # nch[bA] >= nch[bB]
    order = sorted(range(B), key=lambda b: -win[b][1])
    return [(order[i], order[i + 1]) for i in range(0, B, 2)]


def _emit_v_load(nc, vt, vc4, hbase, b, win, nch, veng):
    """DMA batch b's V window (ceil-chunks) into vt cols [hbase, ...)."""
    lo, nv, runs = win[b]
    nload = nch[b] * 128
    if len(runs) == 1:
        # quad/pair fast path: one descriptor covers 4 (or 2) chunks
        col = runs[0][0]
        nquad, rest = divmod(nch[b], 4)
        if nquad:
            dst = vt[:, hbase:hbase + nquad * 4 * DV].rearrange(
                "p (cq f) -> p cq f", f=4 * DV)
            src = vc4[col:col + nquad * 512, :].rearrange(
                "(cq q) f -> q cq f", q=512)[0:128]
            veng.dma_start(dst, src)
        c0 = nquad * 4
        if rest >= 2:
            dst = vt[:, hbase + c0 * DV:hbase + (c0 + 2) * DV]
            src = vc4[col + c0 * 128:col + c0 * 128 + 128, 0:2 * DV]
            veng.dma_start(dst, src)
            c0 += 2
            rest -= 2
        if rest:
            veng.dma_start(
                vt[:, hbase + c0 * DV:hbase + (c0 + 1) * DV],
                vc4[col + c0 * 128:col + c0 * 128 + 128, 0:DV],
            )
    else:
        # general multi-run path (non-arange page tables)
        o = 0
        for ri, (col, ln) in enumerate(runs):
            if ri == len(runs) - 1:
                ln += nload - nv
            while ln > 0:
                c, p0 = divmod(o, 128)
                cbase = hbase + c * DV
                if p0 == 0 and ln >= 128:
                    nf = ln // 128
                    if nf > 1:
                        dst = vt[:, hbase:hbase + nch[b] * DV].rearrange(
                            "p (c f) -> p c f", f=DV)[:, c:c + nf, :]
                        src = vc4[col:col + nf * 128, 0:DV].rearrange(
                            "(c p) f -> p c f", p=128)
                    else:
                        dst = vt[:, cbase:cbase + DV]
                        src = vc4[col:col + 128, 0:DV]
                    veng.dma_start(dst, src)
                    adv = nf * 128
                else:
                    seg = min(128 - p0, ln)
                    veng.dma_start(
                        vt[p0:p0 + seg, cbase:cbase + DV],
                        vc4[col:col + seg, 0:DV],
                    )
                    adv = seg
                o += adv
                col += adv
                ln -= adv


def _build_program(win):
    """win: list per batch of (lo, nv, runs). Returns finalized Bacc program.

    The program is shared by all 8 cores (SPMD); only the data differs.
    """
    nc = bacc.Bacc("TRN2", debug=False)

    ktc = nc.dram_tensor("ktc", [D, PL + KPAD], F16, kind="ExternalInput")
    vc4 = nc.dram_tensor("vc4", [PL + VPAD, 4 * DV], F16, kind="ExternalInput")
    qtz = nc.dram_tensor("qtz", [128, B * G], F16, kind="ExternalInput")
    sk = nc.dram_tensor("sk", [G, 1], F32, kind="ExternalInput")
    out = nc.dram_tensor("out", [4, G, 2 * D], F32, kind="ExternalOutput")

    pairs = _pairs_of(win)
    nch = {b: (win[b][1] + 127) // 128 for b in range(B)}
    rem = {b: win[b][1] - (nch[b] - 1) * 128 for b in range(B)}

    with tile.TileContext(nc) as tc:
        with (
            tc.tile_pool(name="const", bufs=1) as cpool,
            tc.tile_pool(name="kt", bufs=4) as kpool,
            tc.tile_pool(name="vv", bufs=4) as vpool,
            tc.tile_pool(name="ee", bufs=4) as epool,
            tc.tile_pool(name="small", bufs=8) as spool,
            tc.tile_pool(name="outp", bufs=4) as opool,
            tc.tile_pool(name="qkp", bufs=4, space="PSUM") as qkpool,
            tc.tile_pool(name="pvp", bufs=2, space="PSUM") as pvpool,
        ):
            qtz_sb = cpool.tile([128, B * G], F16)
            nc.scalar.dma_start(qtz_sb[:], qtz[:])
            sk_sb = cpool.tile([G, 1], F32)
            nc.scalar.dma_start(sk_sb[:], sk[:])
            esink = cpool.tile([G, 1], F32)
            nc.scalar.activation(esink[:], sk_sb[:], mybir.ActivationFunctionType.Exp)

            # ---- load phase: all K and V DMAs up front ----
            ktiles, vtiles = [], []
            for pi, (bA, bB) in enumerate(pairs):
                ncp = nch[bA]
                ktile = kpool.tile([128, ncp * 128], F16, tag="ktile")
                # half-grouped V: A slots [0, ncp*DV), B slots [ncp*DV, ...)
                vt = vpool.tile([128, 2 * ncp * DV], F16, tag="vtile")
                ktiles.append(ktile)
                vtiles.append(vt)
                for half, b in ((0, bA), (1, bB)):
                    lo, nv, runs = win[b]
                    nload = nch[b] * 128
                    o = 0
                    for ri, (col, ln) in enumerate(runs):
                        if ri == len(runs) - 1:
                            ln += nload - nv
                        if pi == 0 and len(runs) == 1 and ln > 512:
                            # split so the first chunks' receipt lands sooner
                            nc.sync.dma_start(
                                ktile[half * 64:half * 64 + 64, o:o + 512],
                                ktc[:, col:col + 512],
                            )
                            nc.sync.dma_start(
                                ktile[half * 64:half * 64 + 64, o + 512:o + ln],
                                ktc[:, col + 512:col + ln],
                            )
                        else:
                            nc.sync.dma_start(
                                ktile[half * 64:half * 64 + 64, o:o + ln],
                                ktc[:, col:col + ln],
                            )
                        o += ln
                    _emit_v_load(nc, vt, vc4, half * ncp * DV, b, win, nch,
                                 nc.gpsimd)

            # ---- compute phase ----
            for pi, (bA, bB) in enumerate(pairs):
                ncp = nch[bA]
                ktile, vt = ktiles[pi], vtiles[pi]
                qk = qkpool.tile([128, ncp * 2 * G], F32)
                for c in range(ncp):
                    nc.tensor.matmul(
                        qk[:, c * 2 * G:(c + 1) * 2 * G],
                        ktile[:, c * 128:(c + 1) * 128],
                        qtz_sb[:, pi * 2 * G:(pi + 1) * 2 * G],
                        start=True, stop=True, skip_group_check=True,
                    )
                e = epool.tile([128, ncp * 2 * G], F16)
                nc.scalar.activation(
                    e[:], qk[:], mybir.ActivationFunctionType.Exp,
                    scale=1.0 / 8.0,
                )
                # P @ [V|1] with 4-wide stationaries: batch A accumulates in
                # pv[0:4, 0:65], batch B in pv[0:4, 65:130]; tail chunks
                # slice the row count to the window
                nA, nB = nch[bA], nch[bB]
                pvA = pvpool.tile([G, DV], F32, tag="pvA", bufs=2)
                pvB = pvpool.tile([G, DV], F32, tag="pvB", bufs=2)
                for c in range(ncp):
                    ra = 128 if (c < nA - 1 or rem[bA] == 128) else rem[bA]
                    nc.tensor.matmul(
                        pvA[:],
                        e[0:ra, c * 2 * G:c * 2 * G + G],
                        vt[0:ra, c * DV:(c + 1) * DV],
                        start=(c == 0), stop=(c == ncp - 1),
                        skip_group_check=True,
                    )
                    if c <= nB - 1:
                        rb = 128 if (c < nB - 1 or rem[bB] == 128) else rem[bB]
                        nc.tensor.matmul(
                            pvB[:],
                            e[0:rb, c * 2 * G + G:(c + 1) * 2 * G],
                            vt[0:rb, (ncp + c) * DV:(ncp + c + 1) * DV],
                            start=(c == 0), stop=(c == nB - 1),
                            skip_group_check=True,
                        )
                pvhalves = (pvA[:], pvB[:])

                osb = opool.tile([G, 2 * D], F32)
                for half in (0, 1):
                    pvh = pvhalves[half]
                    den = spool.tile([G, 1], F32, tag="den")
                    nc.vector.tensor_add(den[:], pvh[:, D:DV], esink[:])
                    inv = spool.tile([G, 1], F32, tag="inv")
                    nc.vector.reciprocal(inv[:], den[:])
                    nc.vector.tensor_scalar_mul(
                        osb[:, half * D:(half + 1) * D], pvh[:, 0:D], inv[:])
                nc.gpsimd.dma_start(out[pi], osb[:])

    nc.finalize()
    return nc


_PROGRAM_CACHE = {}


def kernel(q_last, k_last, v_last, cache_state, seq_block_ids, start_positions,
           sink):
    global LAST_EXEC_NS

    q_last = np.asarray(q_last, dtype=np.float32)
    k_last = np.asarray(k_last, dtype=np.float32)
    v_last = np.asarray(v_last, dtype=np.float32)
    cache_state = np.asarray(cache_state, dtype=np.float32)
    sbi = np.asarray(seq_block_ids)
    pos = np.asarray(start_positions)
    sink = np.asarray(sink, dtype=np.float32)

    win = [_window_runs(sbi[b], int(pos[b])) for b in range(B)]

    key = (tuple(int(p) for p in pos), sbi.tobytes())
    if key not in _PROGRAM_CACHE:
        _PROGRAM_CACHE.clear()
        _PROGRAM_CACHE[key] = _build_program(win)
    nc = _PROGRAM_CACHE[key]

    pairs = _pairs_of(win)

    # decode-token scatter targets (page-linear)
    scat = [int(sbi[b, int(pos[b]) // S]) * S + int(pos[b]) % S for b in range(B)]

    in_maps = []
    for h in range(Hk):
        kt = np.zeros((D, PL + KPAD), dtype=F16_NP)
        kt[:, :PL] = cache_state[:, 0, h].reshape(PL, D).T.astype(F16_NP)
        vcv = np.zeros((PL + VPAD, DV), dtype=F16_NP)
        vcv[:PL, :D] = cache_state[:, 1, h].reshape(PL, D).astype(F16_NP)
        vcv[:, D] = 1.0
        for b in range(B):
            kt[:, scat[b]] = k_last[b, 0, h].astype(F16_NP)
            vcv[scat[b], :D] = v_last[b, 0, h].astype(F16_NP)
        vc4 = np.zeros((PL + VPAD, 4 * DV), dtype=F16_NP)
        vc4[:, 0:DV] = vcv
        vc4[:-128, DV:2 * DV] = vcv[128:]
        vc4[:-256, 2 * DV:3 * DV] = vcv[256:]
        vc4[:-384, 3 * DV:4 * DV] = vcv[384:]
        qz = np.zeros((128, B * G), dtype=F16_NP)
        for pi, (bA, bB) in enumerate(pairs):
            qz[0:D, pi * 2 * G:pi * 2 * G + G] = \
                q_last[bA, 0, h * G:(h + 1) * G].T.astype(F16_NP)
            qz[D:128, pi * 2 * G + G:(pi + 1) * 2 * G] = \
                q_last[bB, 0, h * G:(h + 1) * G].T.astype(F16_NP)
        skv = np.ascontiguousarray(sink[h * G:(h + 1) * G].reshape(G, 1))
        in_maps.append({"ktc": kt, "vc4": vc4, "qtz": qz, "sk": skv})

    prof = os.environ.get("BASS_KERNEL_PROF", "") == "1"
    kwargs = {}
    if prof:
        kwargs = dict(trace=True, trace_cores=list(range(NCORES)))
        tdir = os.environ.get("BASS_KERNEL_PROF_DIR")
        if tdir:
            kwargs["tmpdir"] = tdir
    res = run_bass_kernel_spmd(nc, in_maps, list(range(NCORES)), **kwargs)
    if prof:
        LAST_EXEC_NS = res.exec_time_ns

    full = np.empty((B, 1, Hq, D), dtype=np.float32)
    for h in range(Hk):
        o = res.results[h]["out"]          # [4, G, 2*D]
        for pi, (bA, bB) in enumerate(pairs):
            full[bA, 0, h * G:(h + 1) * G, :] = o[pi, :, 0:D]
            full[bB, 0, h * G:(h + 1) * G, :] = o[pi, :, D:2 * D]
    return full


# revision 24
# speedup vs baseline: 1.1665x; 1.1028x over previous
"""Bass/Trainium2 kernel for nn_DecodeWrapperEager (sparse paged-attention decode).

Problem (hardcoded): B=8, Hq=32, Hk=8, D=64, S=32, NB=128, T=4096, W=1024.
One decode step of GQA attention with a paged KV cache, sliding window W and
a sink logit.  Output: [B, 1, Hq, D] float32.

Sharding: KV heads across the 8 cores (tensor-parallel).  Each core receives
its head's cache in two layouts staged on host (fp16: same PE speed and DMA
bytes as bf16, 8x the mantissa):
  - ktc [64, PL+pad]   : K^T, column = page*32 + slot (page-linear), k_last
                         scattered at the decode position.
  - vc4 [PL+pad, 260]  : row t = [Vaug_t | Vaug_{t+128} | Vaug_{t+256} |
                         Vaug_{t+384}] where Vaug_t = [V_t | 1] (the fused
                         ones-column makes the softmax denominator fall out
                         of the P @ [V|1] matmul).  Position-quadded so one
                         DMA descriptor covers four 128-chunks (520B runs).
Per-batch sliding-window offsets are identical on every core, so they are
baked into the single SPMD program; the device gathers only the window pages.
Batches are packed in pairs onto the 128-partition contraction dim (head dim
is 64) so the PE array and all 16 DMA ports are fully used.  Windows are
loaded to ceil-of-128 chunks (overreads land in padded/neighbor pages); the
out-of-window tail rows are excluded by partition-sliced tail matmuls.  The
two P@[V|1] matmuls per chunk use 4-wide stationaries so both batches'
accumulators live on partitions 0-3 (compute engines need 32-aligned
partition starts, so an [8,130] result would need a DMA round trip).
Engine assignment is by pipeline stage, and all loads are emitted before all
compute so every queue is monotonic in dependency time: sync=K,
gpsimd=V(A half)+outputs, scalar=V(B half)+exp, vector=epilogue, PE=matmuls.
"""

import os
import numpy as np

import concourse.bacc as bacc
import concourse.tile as tile
import concourse.mybir as mybir
from concourse.bass_utils import run_bass_kernel_spmd

F32 = mybir.dt.float32
F16 = mybir.dt.float16
F16_NP = np.float16

B, Hq, Hk, D = 8, 32, 8, 64
S, NB = 32, 128
T = NB * S          # 4096 positions per sequence
P = B * NB          # 1024 cache pages
W = 1024            # sliding window
G = Hq // Hk        # 4 query heads per kv head
NCORES = 8
PL = P * S          # 32768 page-linear positions
KPAD = 128          # ceil-chunk overread slack past the last page
VPAD = 1024         # slack for the position-shifted right slots + overread
DV = D + 1          # V row with fused ones-column

LAST_EXEC_NS = None


def _window_runs(sbi_b, pos):
    """Contiguous runs (col, len) in page-linear space covering [lo, pos]."""
    lo = max(0, pos - W + 1)
    nv = pos + 1 - lo
    runs = []
    t = lo
    while t <= pos:
        blk = t // S
        s0 = t % S
        s1 = min(S - 1, pos - blk * S)
        col = int(sbi_b[blk]) * S + s0
        ln = s1 - s0 + 1
        if runs and runs[-1][0] + runs[-1][1] == col:
            runs[-1][1] += ln
        else:
            runs.append([col, ln])
        t = blk * S + s1 + 1
    return lo, nv, [(c, l) for c, l in runs]


def _pairs_of(win):
    # larger window first in each pair: tail-chunk matmuls rely on
    # nch[bA] >= nch[bB]
    order = sorted(range(B), key=lambda b: -win[b][1])
    return [(order[i], order[i + 1]) for i in range(0, B, 2)]


def _emit_v_load(nc, vt, vc4, hbase, b, win, nch, veng):
    """DMA batch b's V window (ceil-chunks) into vt cols [hbase, ...)."""
    lo, nv, runs = win[b]
    nload = nch[b] * 128
    if len(runs) == 1:
        # quad/pair fast path: one descriptor covers 4 (or 2) chunks
        col = runs[0][0]
        nquad, rest = divmod(nch[b], 4)
        if nquad:
            dst = vt[:, hbase:hbase + nquad * 4 * DV].rearrange(
                "p (cq f) -> p cq f", f=4 * DV)
            src = vc4[col:col + nquad * 512, :].rearrange(
                "(cq q) f -> q cq f", q=512)[0:128]
            veng.dma_start(dst, src)
        c0 = nquad * 4
        if rest >= 2:
            dst = vt[:, hbase + c0 * DV:hbase + (c0 + 2) * DV]
            src = vc4[col + c0 * 128:col + c0 * 128 + 128, 0:2 * DV]
            veng.dma_start(dst, src)
            c0 += 2
            rest -= 2
        if rest:
            veng.dma_start(
                vt[:, hbase + c0 * DV:hbase + (c0 + 1) * DV],
                vc4[col + c0 * 128:col + c0 * 128 + 128, 0:DV],
            )
    else:
        # general multi-run path (non-arange page tables)
        o = 0
        for ri, (col, ln) in enumerate(runs):
            if ri == len(runs) - 1:
                ln += nload - nv
            while ln > 0:
                c, p0 = divmod(o, 128)
                cbase = hbase + c * DV
                if p0 == 0 and ln >= 128:
                    nf = ln // 128
                    if nf > 1:
                        dst = vt[:, hbase:hbase + nch[b] * DV].rearrange(
                            "p (c f) -> p c f", f=DV)[:, c:c + nf, :]
                        src = vc4[col:col + nf * 128, 0:DV].rearrange(
                            "(c p) f -> p c f", p=128)
                    else:
                        dst = vt[:, cbase:cbase + DV]
                        src = vc4[col:col + 128, 0:DV]
                    veng.dma_start(dst, src)
                    adv = nf * 128
                else:
                    seg = min(128 - p0, ln)
                    veng.dma_start(
                        vt[p0:p0 + seg, cbase:cbase + DV],
                        vc4[col:col + seg, 0:DV],
                    )
                    adv = seg
                o += adv
                col += adv
                ln -= adv


def _build_program(win):
    """win: list per batch of (lo, nv, runs). Returns finalized Bacc program.

    The program is shared by all 8 cores (SPMD); only the data differs.
    """
    nc = bacc.Bacc("TRN2", debug=False)

    ktc = nc.dram_tensor("ktc", [D, PL + KPAD], F16, kind="ExternalInput")
    vc4 = nc.dram_tensor("vc4", [PL + VPAD, 4 * DV], F16, kind="ExternalInput")
    qtz = nc.dram_tensor("qtz", [128, B * G], F16, kind="ExternalInput")
    sk = nc.dram_tensor("sk", [G, 1], F32, kind="ExternalInput")
    out = nc.dram_tensor("out", [4, G, 2 * D], F32, kind="ExternalOutput")

    pairs = _pairs_of(win)
    nch = {b: (win[b][1] + 127) // 128 for b in range(B)}
    rem = {b: win[b][1] - (nch[b] - 1) * 128 for b in range(B)}

    with tile.TileContext(nc) as tc:
        with (
            tc.tile_pool(name="const", bufs=1) as cpool,
            tc.tile_pool(name="kt", bufs=4) as kpool,
            tc.tile_pool(name="vv", bufs=4) as vpool,
            tc.tile_pool(name="ee", bufs=4) as epool,
            tc.tile_pool(name="small", bufs=8) as spool,
            tc.tile_pool(name="outp", bufs=4) as opool,
            tc.tile_pool(name="qkp", bufs=4, space="PSUM") as qkpool,
            tc.tile_pool(name="pvp", bufs=2, space="PSUM") as pvpool,
        ):
            qtz_sb = cpool.tile([128, B * G], F16)
            nc.scalar.dma_start(qtz_sb[:], qtz[:])
            sk_sb = cpool.tile([G, 1], F32)
            nc.scalar.dma_start(sk_sb[:], sk[:])
            esink = cpool.tile([G, 1], F32)
            nc.scalar.activation(esink[:], sk_sb[:], mybir.ActivationFunctionType.Exp)

            # ---- load phase: all K and V DMAs up front ----
            ktiles, vtiles = [], []
            for pi, (bA, bB) in enumerate(pairs):
                ncp = nch[bA]
                ktile = kpool.tile([128, ncp * 128], F16, tag="ktile")
                # half-grouped V: A slots [0, ncp*DV), B slots [ncp*DV, ...)
                vt = vpool.tile([128, 2 * ncp * DV], F16, tag="vtile")
                ktiles.append(ktile)
                vtiles.append(vt)
                for half, b in ((0, bA), (1, bB)):
                    lo, nv, runs = win[b]
                    nload = nch[b] * 128
                    o = 0
                    for ri, (col, ln) in enumerate(runs):
                        if ri == len(runs) - 1:
                            ln += nload - nv
                        nc.sync.dma_start(
                            ktile[half * 64:half * 64 + 64, o:o + ln],
                            ktc[:, col:col + ln],
                        )
                        o += ln
                    _emit_v_load(nc, vt, vc4, half * ncp * DV, b, win, nch,
                                 nc.gpsimd)

            # ---- compute phase ----
            for pi, (bA, bB) in enumerate(pairs):
                ncp = nch[bA]
                ktile, vt = ktiles[pi], vtiles[pi]
                qk = qkpool.tile([128, ncp * 2 * G], F32)
                for c in range(ncp):
                    nc.tensor.matmul(
                        qk[:, c * 2 * G:(c + 1) * 2 * G],
                        ktile[:, c * 128:(c + 1) * 128],
                        qtz_sb[:, pi * 2 * G:(pi + 1) * 2 * G],
                        start=True, stop=True, skip_group_check=True,
                    )
                e = epool.tile([128, ncp * 2 * G], F16)
                nc.scalar.activation(
                    e[:], qk[:], mybir.ActivationFunctionType.Exp,
                    scale=1.0 / 8.0,
                )
                # P @ [V|1] with 4-wide stationaries: batch A accumulates in
                # pv[0:4, 0:65], batch B in pv[0:4, 65:130]; tail chunks
                # slice the row count to the window
                nA, nB = nch[bA], nch[bB]
                pvA = pvpool.tile([G, DV], F32, tag="pvA", bufs=2)
                pvB = pvpool.tile([G, DV], F32, tag="pvB", bufs=2)
                for c in range(ncp):
                    ra = 128 if (c < nA - 1 or rem[bA] == 128) else rem[bA]
                    nc.tensor.matmul(
                        pvA[:],
                        e[0:ra, c * 2 * G:c * 2 * G + G],
                        vt[0:ra, c * DV:(c + 1) * DV],
                        start=(c == 0), stop=(c == ncp - 1),
                        skip_group_check=True,
                    )
                    if c <= nB - 1:
                        rb = 128 if (c < nB - 1 or rem[bB] == 128) else rem[bB]
                        nc.tensor.matmul(
                            pvB[:],
                            e[0:rb, c * 2 * G + G:(c + 1) * 2 * G],
                            vt[0:rb, (ncp + c) * DV:(ncp + c + 1) * DV],
                            start=(c == 0), stop=(c == nB - 1),
                            skip_group_check=True,
                        )
                pvhalves = (pvA[:], pvB[:])

                osb = opool.tile([G, 2 * D], F32)
                for half in (0, 1):
                    pvh = pvhalves[half]
                    den = spool.tile([G, 1], F32, tag="den")
                    nc.vector.tensor_add(den[:], pvh[:, D:DV], esink[:])
                    inv = spool.tile([G, 1], F32, tag="inv")
                    nc.vector.reciprocal(inv[:], den[:])
                    nc.vector.tensor_scalar_mul(
                        osb[:, half * D:(half + 1) * D], pvh[:, 0:D], inv[:])
                nc.gpsimd.dma_start(out[pi], osb[:])

    nc.finalize()
    return nc


_PROGRAM_CACHE = {}


def kernel(q_last, k_last, v_last, cache_state, seq_block_ids, start_positions,
           sink):
    global LAST_EXEC_NS

    q_last = np.asarray(q_last, dtype=np.float32)
    k_last = np.asarray(k_last, dtype=np.float32)
    v_last = np.asarray(v_last, dtype=np.float32)
    cache_state = np.asarray(cache_state, dtype=np.float32)
    sbi = np.asarray(seq_block_ids)
    pos = np.asarray(start_positions)
    sink = np.asarray(sink, dtype=np.float32)

    win = [_window_runs(sbi[b], int(pos[b])) for b in range(B)]

    key = (tuple(int(p) for p in pos), sbi.tobytes())
    if key not in _PROGRAM_CACHE:
        _PROGRAM_CACHE.clear()
        _PROGRAM_CACHE[key] = _build_program(win)
    nc = _PROGRAM_CACHE[key]

    pairs = _pairs_of(win)

    # decode-token scatter targets (page-linear)
    scat = [int(sbi[b, int(pos[b]) // S]) * S + int(pos[b]) % S for b in range(B)]

    in_maps = []
    for h in range(Hk):
        kt = np.zeros((D, PL + KPAD), dtype=F16_NP)
        kt[:, :PL] = cache_state[:, 0, h].reshape(PL, D).T.astype(F16_NP)
        vcv = np.zeros((PL + VPAD, DV), dtype=F16_NP)
        vcv[:PL, :D] = cache_state[:, 1, h].reshape(PL, D).astype(F16_NP)
        vcv[:, D] = 1.0
        for b in range(B):
            kt[:, scat[b]] = k_last[b, 0, h].astype(F16_NP)
            vcv[scat[b], :D] = v_last[b, 0, h].astype(F16_NP)
        vc4 = np.zeros((PL + VPAD, 4 * DV), dtype=F16_NP)
        vc4[:, 0:DV] = vcv
        vc4[:-128, DV:2 * DV] = vcv[128:]
        vc4[:-256, 2 * DV:3 * DV] = vcv[256:]
        vc4[:-384, 3 * DV:4 * DV] = vcv[384:]
        qz = np.zeros((128, B * G), dtype=F16_NP)
        for pi, (bA, bB) in enumerate(pairs):
            qz[0:D, pi * 2 * G:pi * 2 * G + G] = \
                q_last[bA, 0, h * G:(h + 1) * G].T.astype(F16_NP)
            qz[D:128, pi * 2 * G + G:(pi + 1) * 2 * G] = \
                q_last[bB, 0, h * G:(h + 1) * G].T.astype(F16_NP)
        skv = np.ascontiguousarray(sink[h * G:(h + 1) * G].reshape(G, 1))
        in_maps.append({"ktc": kt, "vc4": vc4, "qtz": qz, "sk": skv})

    prof = os.environ.get("BASS_KERNEL_PROF", "") == "1"
    kwargs = {}
    if prof:
        kwargs = dict(trace=True, trace_cores=list(range(NCORES)))
        tdir = os.environ.get("BASS_KERNEL_PROF_DIR")
        if tdir:
            kwargs["tmpdir"] = tdir
    res = run_bass_kernel_spmd(nc, in_maps, list(range(NCORES)), **kwargs)
    if prof:
        LAST_EXEC_NS = res.exec_time_ns

    full = np.empty((B, 1, Hq, D), dtype=np.float32)
    for h in range(Hk):
        o = res.results[h]["out"]          # [4, G, 2*D]
        for pi, (bA, bB) in enumerate(pairs):
            full[bA, 0, h * G:(h + 1) * G, :] = o[pi, :, 0:D]
            full[bB, 0, h * G:(h + 1) * G, :] = o[pi, :, D:2 * D]
    return full
